# revision 1
# baseline (speedup 1.0000x reference)
"""Trainium2 Bass kernel for nn_DoubleStream_Expert (dense double-stream DiT block).

Sharding (8 cores, no collectives): core c -> batch b = c//4, rank r = c%4.
Each core computes the full K/V projections for its batch (2048 tokens, both
streams), but Q / attention / out-proj / MLP only for its own 512 tokens.
Host slices inputs per core and reassembles the two output streams.

Token chunks are fed in a per-core "slot" order (own chunk, other chunk of my
stream, the two chunks of the other stream) so the SPMD program is identical
across cores; attention is permutation-invariant in keys, and RoPE tables are
permuted on the host to match.

Head dims are padded 96->128 with the rotary halves at rows 0..47 / 64..111,
making rotate_half a uniform +-64 partition move (32-aligned starts, written
via shifted-output ops). Padded weight columns are zero.

Precision: fp32r matmuls (full PE rate at N>=256) for Q/K projections and the
out-projection; bf16 for K/Q storage + rope, probs x V, and the MLP; fp32 for
all softmax statistics, norms and residuals. Softmax needs no running max:
QK-norm bounds |logits| <= max(qk_scale)^2/sqrt(dh).
"""

import numpy as np

import concourse.bass as bass  # noqa: F401
import concourse.mybir as mybir
import concourse.tile as tile
from concourse import bacc
from concourse.bass_utils import run_bass_kernel_spmd
from concourse.masks import make_identity

try:
    import ml_dtypes
    _BF16 = ml_dtypes.bfloat16
except ImportError:  # pragma: no cover
    _BF16 = np.float32

F32 = mybir.dt.float32
F32R = mybir.dt.float32r
BF16 = mybir.dt.bfloat16
AF = mybir.ActivationFunctionType
ALU = mybir.AluOpType

B, T, D, H, DH, MLPD = 2, 1024, 768, 8, 96, 3072
N = 2 * T
NC = 8
CH = 512
KT = D // 128        # 6
MT2 = MLPD // 128    # 24
PH = 128
VW = H * 97          # 776
EPS = 1e-6

_ROWS_LO = np.arange(0, 48)
_ROWS_HI = np.arange(64, 112)

_CACHED = {}


def _bc3(ap2d, nh):
    """[P, C] -> [P, nh, C] stride-0 broadcast over a middle axis."""
    return ap2d.unsqueeze(1).broadcast_to([ap2d.shape[0], nh, ap2d.shape[1]])


def _build():
    if "nc" in _CACHED:
        return _CACHED["nc"]

    nc = bacc.Bacc("TRN2", target_bir_lowering=False, debug=False, num_devices=NC)

    def din(name, shape, dt=F32R):
        return nc.dram_tensor(name, list(shape), dt, kind="ExternalInput").ap()

    x_own = din("x_own", [CH, D], F32)
    x_rest = din("x_rest", [3, CH, D], F32)
    x_own2 = din("x_own2", [CH, D], F32)               # second copy for the residual
    p_my = din("p_my", [1, 1024], BF16)
    mod_w1 = din("mod_w1", [1024, 512], BF16)
    mod_b1 = din("mod_b1", [128, 4], F32)
    mod_w2m = din("mod_w2m", [512, 6 * D], BF16)  # ms_my mh_my ms_ot mh_ot m3s m3h
    mod_b2m = din("mod_b2m", [128, 36], F32)
    mod_w2g = din("mod_w2g", [512, 2 * D], BF16)  # g_my, m3g
    mod_b2g = din("mod_b2g", [1, 2 * D], F32)
    norm1_my = din("norm1_my", [128, KT], F32)
    norm1_ot = din("norm1_ot", [128, KT], F32)
    norm2_my = din("norm2_my", [128, KT], F32)
    wq = din("wq", [D, H * PH])
    bq = din("bq", [128, H], F32)
    wk_my = din("wk_my", [D, H * PH])
    wk_ot = din("wk_ot", [D, H * PH])
    bk_my = din("bk_my", [128, H], F32)
    bk_ot = din("bk_ot", [128, H], F32)
    wv_my = din("wv_my", [D, VW], BF16)
    wv_ot = din("wv_ot", [D, VW], BF16)
    bv_my = din("bv_my", [1, VW], F32)
    bv_ot = din("bv_ot", [1, VW], F32)
    cos_t = din("cos_t", [128, N], BF16)
    sin_t = din("sin_t", [128, N], BF16)
    qk_s2 = din("qk_s2", [128, 1], F32)
    wo = din("wo", [96, H * D], BF16)
    ob_g = din("ob_g", [1, D], F32)
    w1 = din("w1", [D, MLPD], BF16)
    b1c = din("b1c", [128, MT2], F32)
    w2 = din("w2", [MLPD, D], BF16)
    b2r = din("b2r", [1, D], F32)

    my_out = nc.dram_tensor("my_out", [CH, D], F32, kind="ExternalOutput").ap()

    with tile.TileContext(nc) as tc:
        persist_cm = tc.tile_pool(name="persist", bufs=1)
        pp = persist_cm.__enter__()

        ident = pp.tile([128, 128], F32, name="ident")
        make_identity(nc, ident[:])
        mod_l2 = pp.tile([128, 36], F32, name="mod_l2")
        g_my_bc = pp.tile([128, D], F32, name="g_my_bc")
        m3g_bc = pp.tile([128, D], F32, name="m3g_bc")
        ob_bc = pp.tile([128, D], F32, name="ob_bc")
        b2_bc = pp.tile([128, D], F32, name="b2_bc")
        w1p = pp.tile([128, KT], F32, name="w1p")
        w2p = pp.tile([128, KT], F32, name="w2p")
        w3p = pp.tile([128, KT], F32, name="w3p")
        s2_sb = pp.tile([128, 1], F32, name="s2_sb")
        bq_sb = pp.tile([128, H], F32, name="bq_sb")
        bkm_sb = pp.tile([128, H], F32, name="bkm_sb")
        bko_sb = pp.tile([128, H], F32, name="bko_sb")
        eps_sb = pp.tile([128, 1], F32, name="eps_sb")
        nc.vector.memset(eps_sb[:], EPS)

        # ---------------- modulation MLP ----------------
        with (
            nc.named_scope("mod"),
            tc.tile_pool(name="modw", bufs=1) as mw,
            tc.tile_pool(name="psm", bufs=1, space="PSUM") as psm,
            tc.tile_pool(name="psg", bufs=2, space="PSUM") as psg,
        ):
            p_sb = mw.tile([128, 8], BF16, name="p_sb")
            nc.sync.dma_start(out=p_sb[:], in_=p_my.rearrange("o (j r) -> r (o j)", r=128))
            ps2 = mw.tile([128, 8], BF16, name="ps2")
            nc.scalar.activation(ps2[:], p_sb[:], AF.Silu)

            w1m_sb = mw.tile([128, 8, 512], BF16, name="w1m_sb")
            nc.sync.dma_start(out=w1m_sb[:], in_=mod_w1.rearrange("(k p) m -> p k m", p=128))
            b1m_sb = mw.tile([128, 4], F32, name="b1m_sb")
            nc.sync.dma_start(out=b1m_sb[:], in_=mod_b1)
            h_ps = psm.tile([128, 4], F32, name="h_ps")
            for mt in range(4):
                for kt in range(8):
                    nc.tensor.matmul(
                        h_ps[:, mt : mt + 1],
                        w1m_sb[:, kt, mt * 128 : (mt + 1) * 128],
                        ps2[:, kt : kt + 1],
                        start=(kt == 0), stop=(kt == 7),
                    )
            h_l2 = mw.tile([128, 4], BF16, name="h_l2")
            for mt in range(4):
                nc.scalar.activation(h_l2[:, mt : mt + 1], h_ps[:, mt : mt + 1],
                                     AF.Silu, bias=b1m_sb[:, mt : mt + 1])

            w2m_sb = mw.tile([128, 4, 6 * D], BF16, name="w2m_sb")
            nc.sync.dma_start(out=w2m_sb[:], in_=mod_w2m.rearrange("(k p) m -> p k m", p=128))
            b2m_sb = mw.tile([128, 36], F32, name="b2m_sb")
            nc.sync.dma_start(out=b2m_sb[:], in_=mod_b2m)
            mod_ps = psm.tile([128, 36], F32, name="mod_ps")
            for mt in range(36):
                for kt in range(4):
                    nc.tensor.matmul(
                        mod_ps[:, mt : mt + 1],
                        w2m_sb[:, kt, mt * 128 : (mt + 1) * 128],
                        h_l2[:, kt : kt + 1],
                        start=(kt == 0), stop=(kt == 3),
                    )
            nc.vector.tensor_add(mod_l2[:], mod_ps[:], b2m_sb[:])

            w2g_sb = mw.tile([128, 4, 2 * D], BF16, name="w2g_sb")
            nc.sync.dma_start(out=w2g_sb[:], in_=mod_w2g.rearrange("(k p) m -> p k m", p=128))
            b2g_sb = mw.tile([1, 2 * D], F32, name="b2g_sb")
            nc.sync.dma_start(out=b2g_sb[:], in_=mod_b2g)
            gates = mw.tile([1, 2 * D], F32, name="gates")
            for nt in range(3):
                g_ps = psg.tile([1, 512], F32, name="g_ps", tag="g_ps")
                for kt in range(4):
                    nc.tensor.matmul(
                        g_ps[:], h_l2[:, kt : kt + 1],
                        w2g_sb[:, kt, nt * 512 : (nt + 1) * 512],
                        start=(kt == 0), stop=(kt == 3),
                    )
                nc.vector.tensor_tensor(gates[:, nt * 512 : (nt + 1) * 512], g_ps[:],
                                        b2g_sb[:, nt * 512 : (nt + 1) * 512], op=ALU.add)
            nc.gpsimd.partition_broadcast(g_my_bc[:], gates[:, 0:D])
            nc.gpsimd.partition_broadcast(m3g_bc[:], gates[:, D : 2 * D])

            obg_sb = mw.tile([1, D], F32, name="obg_sb")
            nc.sync.dma_start(out=obg_sb[:], in_=ob_g)
            nc.gpsimd.partition_broadcast(ob_bc[:], obg_sb[:])
            b2r_sb = mw.tile([1, D], F32, name="b2r_sb")
            nc.sync.dma_start(out=b2r_sb[:], in_=b2r)
            nc.gpsimd.partition_broadcast(b2_bc[:], b2r_sb[:])

            n1my_sb = mw.tile([128, KT], F32, name="n1my_sb")
            n1ot_sb = mw.tile([128, KT], F32, name="n1ot_sb")
            n2my_sb = mw.tile([128, KT], F32, name="n2my_sb")
            nc.sync.dma_start(out=n1my_sb[:], in_=norm1_my)
            nc.sync.dma_start(out=n1ot_sb[:], in_=norm1_ot)
            nc.sync.dma_start(out=n2my_sb[:], in_=norm2_my)
            tmp6 = mw.tile([128, KT], F32, name="tmp6")
            nc.vector.tensor_scalar_add(tmp6[:], mod_l2[:, 0:6], 1.0)
            nc.vector.tensor_mul(w1p[:], n1my_sb[:], tmp6[:])
            tmp6b = mw.tile([128, KT], F32, name="tmp6b")
            nc.vector.tensor_scalar_add(tmp6b[:], mod_l2[:, 12:18], 1.0)
            nc.vector.tensor_mul(w2p[:], n1ot_sb[:], tmp6b[:])
            tmp6c = mw.tile([128, KT], F32, name="tmp6c")
            nc.vector.tensor_scalar_add(tmp6c[:], mod_l2[:, 24:30], 1.0)
            nc.vector.tensor_mul(w3p[:], n2my_sb[:], tmp6c[:])
            nc.sync.dma_start(out=s2_sb[:], in_=qk_s2)
            nc.sync.dma_start(out=bq_sb[:], in_=bq)
            nc.sync.dma_start(out=bkm_sb[:], in_=bk_my)
            nc.sync.dma_start(out=bko_sb[:], in_=bk_ot)

        # ---------------- big persistent activations ----------------
        x1n = pp.tile([128, 4, D], F32R, name="x1n")
        with tc.tile_pool(name="poolA", bufs=1) as pa:
            K_sb = pa.tile([128, H, N], BF16, name="K_sb")
            V_sb = pa.tile([128, N // 128, VW], BF16, name="V_sb")
            Q_sb = pa.tile([128, H, CH], BF16, name="Q_sb")
            cos_sb = pa.tile([128, N], BF16, name="cos_sb")
            sin_sb = pa.tile([128, N], BF16, name="sin_sb")
            nc.sync.dma_start(out=cos_sb[:], in_=cos_t)
            nc.sync.dma_start(out=sin_sb[:], in_=sin_t)

            # ---------------- phase 1: xm + Q/K/V projections + rope ----------------
            with (
                nc.named_scope("proj"),
                tc.tile_pool(name="wkvp", bufs=1) as wkvp,
                tc.tile_pool(name="ph1", bufs=1) as ph1,
                tc.tile_pool(name="ph1b", bufs=2) as ph1b,
                tc.tile_pool(name="psP", bufs=2, space="PSUM") as psP,
                tc.tile_pool(name="psV", bufs=2, space="PSUM") as psV,
                tc.tile_pool(name="psT", bufs=2, space="PSUM") as psT,
            ):
                wk_cur = None
                wv_cur = None
                bv_cur = None
                for sl in range(4):
                    my_stream = sl < 2
                    x_l1 = ph1b.tile([128, 4, D], F32, name="x_l1", tag="x_l1")
                    src = x_own if sl == 0 else x_rest[sl - 1]
                    nc.sync.dma_start(out=x_l1[:], in_=src.rearrange("(t p) c -> p t c", p=128))

                    # rms: xs = x * rstd, in place (stats batched over the 4 tok-tiles)
                    ssq4 = ph1.tile([128, 4], F32, name="ssq4b", tag="ssq4b")
                    for tt in range(4):
                        sq = ph1.tile([128, D], F32, name="sq", tag="sq")
                        nc.scalar.activation(sq[:], x_l1[:, tt, :], AF.Square,
                                             accum_out=ssq4[:, tt : tt + 1])
                    rstd4 = ph1.tile([128, 4], F32, name="rstd4b", tag="rstd4b")
                    nc.scalar.activation(rstd4[:], ssq4[:], AF.Abs_reciprocal_sqrt,
                                         scale=1.0 / D, bias=eps_sb[:])
                    for tt in range(4):
                        nc.vector.tensor_scalar_mul(x_l1[:, tt, :], x_l1[:, tt, :],
                                                    rstd4[:, tt : tt + 1])

                    # transpose + modulate -> xm_l2 (f32r) and a bf16 copy for V
                    xm_l2 = ph1b.tile([128, KT, CH], F32R, name="xm_l2", tag="xm_l2")
                    wsel = w1p if my_stream else w2p
                    hoff = 6 if my_stream else 18
                    for tt in range(4):
                        for ft in range(KT):
                            tp = psT.tile([128, 128], F32, name="tp", tag="tp")
                            nc.tensor.transpose(tp[:], x_l1[:, tt, ft * 128 : (ft + 1) * 128], ident[:])
                            nc.vector.tensor_scalar(
                                xm_l2[:, ft, tt * 128 : (tt + 1) * 128], tp[:],
                                wsel[:, ft : ft + 1], mod_l2[:, hoff + ft : hoff + ft + 1],
                                op0=ALU.mult, op1=ALU.add,
                            )
                    xm_bf = ph1.tile([128, KT, CH], BF16, name="xm_bf", tag="xm_bf")
                    nc.vector.tensor_copy(xm_bf[:], xm_l2[:])

                    # Q projection (own chunk only)
                    if sl == 0:
                        wq_sb = wkvp.tile([128, KT, H * PH], F32R, name="wq_sb", tag="wbig")
                        nc.sync.dma_start(out=wq_sb[:], in_=wq.rearrange("(k p) m -> p k m", p=128))
                        for h in range(H):
                            qp = psP.tile([128, CH], F32, name="qp", tag="qp")
                            for kt in range(KT):
                                nc.tensor.matmul(
                                    qp[:], wq_sb[:, kt, h * PH : (h + 1) * PH],
                                    xm_l2[:, kt, :], start=(kt == 0), stop=(kt == KT - 1),
                                )
                            nc.scalar.activation(Q_sb[:, h, :], qp[:], AF.Identity,
                                                 bias=bq_sb[:, h : h + 1])

                    # K projection
                    if sl in (0, 2):
                        wk_sb = wkvp.tile([128, KT, H * PH], F32R, name="wk_sb", tag="wbig")
                        nc.sync.dma_start(
                            out=wk_sb[:],
                            in_=(wk_my if my_stream else wk_ot).rearrange("(k p) m -> p k m", p=128),
                        )
                        wk_cur = wk_sb
                    bsel = bkm_sb if my_stream else bko_sb
                    for h in range(H):
                        kp = psP.tile([128, CH], F32, name="kp", tag="qp")
                        for kt in range(KT):
                            nc.tensor.matmul(
                                kp[:], wk_cur[:, kt, h * PH : (h + 1) * PH],
                                xm_l2[:, kt, :], start=(kt == 0), stop=(kt == KT - 1),
                            )
                        nc.scalar.activation(K_sb[:, h, sl * CH : (sl + 1) * CH], kp[:],
                                             AF.Identity, bias=bsel[:, h : h + 1])

                    # V projection, direct L1
                    if sl in (0, 2):
                        wv_sb = wkvp.tile([128, KT, VW], BF16, name="wv_sb", tag="wv")
                        nc.sync.dma_start(
                            out=wv_sb[:],
                            in_=(wv_my if my_stream else wv_ot).rearrange("(k p) m -> p k m", p=128),
                        )
                        bv_bc = wkvp.tile([128, VW], F32, name="bv_bc", tag="bv_bc")
                        bv_row = ph1.tile([1, VW], F32, name="bv_row", tag="bv_row")
                        nc.sync.dma_start(out=bv_row[:], in_=(bv_my if my_stream else bv_ot))
                        nc.gpsimd.partition_broadcast(bv_bc[:], bv_row[:])
                        wv_cur = wv_sb
                        bv_cur = bv_bc
                    for tt in range(4):
                        vp1 = psV.tile([128, 512], F32, name="vp1", tag="vp1")
                        vp2 = psV.tile([128, VW - 512], F32, name="vp2", tag="vp2")
                        for kt in range(KT):
                            nc.tensor.matmul(
                                vp1[:], xm_bf[:, kt, tt * 128 : (tt + 1) * 128],
                                wv_cur[:, kt, 0:512], start=(kt == 0), stop=(kt == KT - 1),
                            )
                        for kt in range(KT):
                            nc.tensor.matmul(
                                vp2[:], xm_bf[:, kt, tt * 128 : (tt + 1) * 128],
                                wv_cur[:, kt, 512:VW], start=(kt == 0), stop=(kt == KT - 1),
                            )
                        nc.vector.tensor_tensor(V_sb[:, sl * 4 + tt, 0:512], vp1[:],
                                                bv_cur[:, 0:512], op=ALU.add)
                        nc.vector.tensor_tensor(V_sb[:, sl * 4 + tt, 512:VW], vp2[:],
                                                bv_cur[:, 512:VW], op=ALU.add)

                    # rope on this K chunk (half the heads at a time; +-64 shifted writes)
                    c3 = cos_sb[:, sl * CH : (sl + 1) * CH]
                    s3 = sin_sb[:, sl * CH : (sl + 1) * CH]
                    HG = H // 2
                    for hg in range(2):
                        kr_t = ph1.tile([128, HG, CH], BF16, name="kr_t", tag="kr_t")
                        kr_m = ph1.tile([128, HG, CH], BF16, name="kr_m", tag="kr_m")
                        ksl = K_sb[:, hg * HG : (hg + 1) * HG, sl * CH : (sl + 1) * CH]
                        nc.vector.tensor_tensor(kr_t[:], ksl, _bc3(c3, HG), op=ALU.mult)
                        nc.vector.tensor_tensor(kr_m[0:64], ksl[64:128], _bc3(s3[64:128], HG), op=ALU.mult)
                        nc.vector.tensor_tensor(kr_m[64:128], ksl[0:64], _bc3(s3[0:64], HG), op=ALU.mult)
                        nc.vector.tensor_tensor(ksl[0:64], kr_t[0:64], kr_m[0:64], op=ALU.subtract)
                        nc.vector.tensor_tensor(ksl[64:128], kr_t[64:128], kr_m[64:128], op=ALU.add)

                    if sl == 0:
                        c0 = cos_sb[:, 0:CH]
                        s0 = sin_sb[:, 0:CH]
                        for hg in range(2):
                            qr_t = ph1.tile([128, HG, CH], BF16, name="qr_t", tag="kr_t")
                            qr_m = ph1.tile([128, HG, CH], BF16, name="qr_m", tag="kr_m")
                            qsl = Q_sb[:, hg * HG : (hg + 1) * HG, :]
                            nc.vector.tensor_tensor(qr_t[:], qsl, _bc3(c0, HG), op=ALU.mult)
                            nc.vector.tensor_tensor(qr_m[0:64], qsl[64:128], _bc3(s0[64:128], HG), op=ALU.mult)
                            nc.vector.tensor_tensor(qr_m[64:128], qsl[0:64], _bc3(s0[0:64], HG), op=ALU.mult)
                            nc.vector.tensor_tensor(qsl[0:64], qr_t[0:64], qr_m[0:64], op=ALU.subtract)
                            nc.vector.tensor_tensor(qsl[64:128], qr_t[64:128], qr_m[64:128], op=ALU.add)

            # ---------------- phases 2+3: qk-norm, attention, out-proj, residual ----------------
            with (
                tc.tile_pool(name="ph2", bufs=2) as ph2,
                tc.tile_pool(name="ph2s", bufs=1) as ph2s,
                tc.tile_pool(name="ph3w", bufs=1) as ph3w,
            ):
                attnn = ph2s.tile([96, H, CH], BF16, name="attnn")
                with (
                    nc.named_scope("attn"),
                    tc.tile_pool(name="psK", bufs=2, space="PSUM") as psK,
                                        tc.tile_pool(name="psS", bufs=2, space="PSUM") as psS,
                    tc.tile_pool(name="psPV", bufs=2, space="PSUM") as psPV,
                ):
                    ones_bf = ph2s.tile([128, 1], BF16, name="ones_bf")
                    nc.vector.memset(ones_bf[:], 1.0)
                    ones = ph2s.tile([128, 1], F32, name="ones")
                    nc.vector.memset(ones[:], 1.0)

                    # rk_all[kt-token, h*16+kt2] = 1/(sqrt(dh)*|k|), per-partition layout
                    rk_all = ph2s.tile([128, H * 16], F32, name="rk_all")
                    rk_ps = psK.tile([128, H * 16], F32, name="rk_ps", tag="rk_ps")
                    for h in range(H):
                        ksq = ph2.tile([128, N], BF16, name="ksq", tag="ksq")
                        nc.vector.tensor_mul(ksq[:], K_sb[:, h, :], K_sb[:, h, :])
                        for kt2 in range(16):
                            nc.tensor.matmul(
                                rk_ps[:, h * 16 + kt2 : h * 16 + kt2 + 1],
                                ksq[:, kt2 * 128 : (kt2 + 1) * 128],
                                ones_bf[:], start=True, stop=True,
                            )
                    nc.scalar.activation(rk_all[:], rk_ps[:], AF.Abs_reciprocal_sqrt,
                                         scale=float(DH), bias=eps_sb[:])

                    # q_hat = q * s2 * (1/|q|)
                    for h in range(H):
                        qsq = ph2.tile([128, CH], BF16, name="qsq", tag="qsq")
                        nc.vector.tensor_mul(qsq[:], Q_sb[:, h, :], Q_sb[:, h, :])
                        rq_ps = psK.tile([1, CH], F32, name="rq_ps", tag="rq_ps")
                        nc.tensor.matmul(rq_ps[:], ones_bf[:], qsq[:], start=True, stop=True)
                        rq_bf = ph2.tile([1, CH], BF16, name="rq_bf", tag="rq_bf")
                        nc.scalar.activation(rq_bf[:], rq_ps[:], AF.Abs_reciprocal_sqrt,
                                             bias=eps_sb[0:1, :])
                        rq_bc = ph2.tile([128, CH], BF16, name="rq_bc", tag="rq_bc")
                        nc.gpsimd.partition_broadcast(rq_bc[:], rq_bf[:])
                        nc.vector.scalar_tensor_tensor(
                            Q_sb[:, h, :], Q_sb[:, h, :], s2_sb[:], rq_bc[:],
                            op0=ALU.mult, op1=ALU.mult,
                        )

                    for h in range(H):
                        pv = psPV.tile([128, CH], F32, name="pv", tag="pv")
                        for kt2 in range(16):
                            sps = psS.tile([128, CH], F32, name="sps", tag="sps")
                            nc.tensor.matmul(
                                sps[:], K_sb[:, h, kt2 * 128 : (kt2 + 1) * 128],
                                Q_sb[:, h, :], start=True, stop=True,
                            )
                            pt = ph2.tile([128, CH], BF16, name="pt", tag="pt")
                            nc.scalar.activation(pt[:], sps[:], AF.Exp,
                                                 scale=rk_all[:, h * 16 + kt2 : h * 16 + kt2 + 1])
                            nc.tensor.matmul(
                                pv[0:97, :], V_sb[:, kt2, h * 97 : (h + 1) * 97],
                                pt[:], start=(kt2 == 0), stop=(kt2 == 15),
                            )
                        rs2 = ph2.tile([1, CH], F32, name="rs2", tag="rs2")
                        nc.scalar.activation(rs2[:], pv[96:97, :], AF.Square)
                        rs = ph2.tile([1, CH], F32, name="rs", tag="rs")
                        nc.scalar.activation(rs[:], rs2[:], AF.Abs_reciprocal_sqrt)
                        rs_bc = ph2.tile([96, CH], F32, name="rs_bc", tag="rs_bc")
                        nc.gpsimd.partition_broadcast(rs_bc[:], rs[:], channels=96)
                        nc.vector.tensor_tensor(attnn[:, h, :], pv[0:96, :], rs_bc[:], op=ALU.mult)

                # out-proj + residual
                with (
                    nc.named_scope("oproj"),
                    tc.tile_pool(name="psO", bufs=2, space="PSUM") as psO,
                ):
                    wo_sb = ph3w.tile([96, H, D], BF16, name="wo_sb")
                    nc.sync.dma_start(out=wo_sb[:], in_=wo.rearrange("p (h m) -> p h m", h=H))
                    xo_l1 = ph3w.tile([128, 4, D], F32, name="xo_l1")
                    nc.sync.dma_start(out=xo_l1[:], in_=x_own2.rearrange("(t p) c -> p t c", p=128))
                    for qt in range(4):
                        op1 = psO.tile([128, 512], F32, name="op1", tag="op1")
                        op2 = psO.tile([128, D - 512], F32, name="op2", tag="op2")
                        for h in range(H):
                            nc.tensor.matmul(
                                op1[:], attnn[:, h, qt * 128 : (qt + 1) * 128],
                                wo_sb[:, h, 0:512], start=(h == 0), stop=(h == H - 1),
                            )
                        for h in range(H):
                            nc.tensor.matmul(
                                op2[:], attnn[:, h, qt * 128 : (qt + 1) * 128],
                                wo_sb[:, h, 512:D], start=(h == 0), stop=(h == H - 1),
                            )
                        t1 = ph2.tile([128, D], F32, name="t1", tag="t1")
                        nc.vector.tensor_tensor(t1[:, 0:512], op1[:], ob_bc[:, 0:512], op=ALU.add)
                        nc.vector.tensor_tensor(t1[:, 512:D], op2[:], ob_bc[:, 512:D], op=ALU.add)
                        nc.vector.tensor_mul(t1[:], t1[:], g_my_bc[:])
                        nc.vector.tensor_tensor(x1n[:, qt, :], t1[:], xo_l1[:, qt, :], op=ALU.add)

        # ---------------- phase 4: norm2 + MLP + final ----------------
        with (
            nc.named_scope("mlp"),
            tc.tile_pool(name="ph4", bufs=2) as ph4,
            tc.tile_pool(name="mlpw", bufs=1) as mlpw,
            tc.tile_pool(name="psM", bufs=2, space="PSUM") as psM,
            tc.tile_pool(name="psM2", bufs=2, space="PSUM") as psM2,
            tc.tile_pool(name="psT2", bufs=2, space="PSUM") as psT2,
        ):
            w1_sb = mlpw.tile([128, KT, MLPD], BF16, name="w1_sb")
            nc.sync.dma_start(out=w1_sb[:], in_=w1.rearrange("(k p) m -> p k m", p=128))
            w2_sb = mlpw.tile([128, MT2, D], BF16, name="w2_sb")
            nc.sync.dma_start(out=w2_sb[:], in_=w2.rearrange("(k p) m -> p k m", p=128))
            b1_sb = mlpw.tile([128, MT2], F32, name="b1_sb")
            nc.sync.dma_start(out=b1_sb[:], in_=b1c)

            xn_l2 = mlpw.tile([128, KT, CH], BF16, name="xn_l2")
            ssq4m = ph4.tile([128, 4], F32, name="ssq4m", tag="ssq4m")
            for tt in range(4):
                sq = ph4.tile([128, D], F32, name="sq4", tag="sq4")
                nc.scalar.activation(sq[:], x1n[:, tt, :], AF.Square,
                                     accum_out=ssq4m[:, tt : tt + 1])
            rstd4m = ph4.tile([128, 4], F32, name="rstd4m", tag="rstd4m")
            nc.scalar.activation(rstd4m[:], ssq4m[:], AF.Abs_reciprocal_sqrt,
                                 scale=1.0 / D, bias=eps_sb[:])
            for tt in range(4):
                xs = ph4.tile([128, D], F32, name="xs4", tag="xs4")
                nc.vector.tensor_scalar_mul(xs[:], x1n[:, tt, :], rstd4m[:, tt : tt + 1])
                for ft in range(KT):
                    tp = psT2.tile([128, 128], F32, name="tp2", tag="tp2")
                    nc.tensor.transpose(tp[:], xs[:, ft * 128 : (ft + 1) * 128], ident[:])
                    nc.vector.tensor_scalar(
                        xn_l2[:, ft, tt * 128 : (tt + 1) * 128], tp[:],
                        w3p[:, ft : ft + 1], mod_l2[:, 30 + ft : 30 + ft + 1],
                        op0=ALU.mult, op1=ALU.add,
                    )

            h_bf = mlpw.tile([128, MT2, CH], BF16, name="h_bf")
            for mt in range(MT2):
                fp = psM.tile([128, CH], F32, name="fp", tag="fp")
                for kt in range(KT):
                    nc.tensor.matmul(
                        fp[:], w1_sb[:, kt, mt * 128 : (mt + 1) * 128],
                        xn_l2[:, kt, :], start=(kt == 0), stop=(kt == KT - 1),
                    )
                nc.scalar.activation(h_bf[:, mt, :], fp[:], AF.Gelu,
                                     bias=b1_sb[:, mt : mt + 1])

            out_f = mlpw.tile([128, 4, D], F32, name="out_f")
            for qt in range(4):
                f1 = psM2.tile([128, 512], F32, name="f1", tag="f1")
                f2 = psM2.tile([128, D - 512], F32, name="f2", tag="f2")
                for mt in range(MT2):
                    nc.tensor.matmul(
                        f1[:], h_bf[:, mt, qt * 128 : (qt + 1) * 128],
                        w2_sb[:, mt, 0:512], start=(mt == 0), stop=(mt == MT2 - 1),
                    )
                for mt in range(MT2):
                    nc.tensor.matmul(
                        f2[:], h_bf[:, mt, qt * 128 : (qt + 1) * 128],
                        w2_sb[:, mt, 512:D], start=(mt == 0), stop=(mt == MT2 - 1),
                    )
                t2 = ph4.tile([128, D], F32, name="t2", tag="t2")
                nc.vector.tensor_tensor(t2[:, 0:512], f1[:], b2_bc[:, 0:512], op=ALU.add)
                nc.vector.tensor_tensor(t2[:, 512:D], f2[:], b2_bc[:, 512:D], op=ALU.add)
                nc.vector.tensor_mul(t2[:], t2[:], m3g_bc[:])
                nc.vector.tensor_tensor(out_f[:, qt, :], t2[:], x1n[:, qt, :], op=ALU.add)
            nc.sync.dma_start(out=my_out.rearrange("(t p) c -> p t c", p=128), in_=out_f[:])

        persist_cm.__exit__(None, None, None)


    nc.compile()
    _CACHED["nc"] = nc
    return nc


def _pad_head_cols(w_h, b_h):
    wp = np.zeros((D, PH), np.float32)
    bp = np.zeros((PH,), np.float32)
    wp[:, _ROWS_LO] = w_h[:, 0:48]
    wp[:, _ROWS_HI] = w_h[:, 48:96]
    bp[_ROWS_LO] = b_h[0:48]
    bp[_ROWS_HI] = b_h[48:96]
    return wp, bp


def _prep_core_inputs(c, inp):
    b, r = c // 4, c % 4
    s = 0 if r < 2 else 1
    sub = r % 2

    x1 = np.asarray(inp["x_stream1"], np.float32)
    x2 = np.asarray(inp["x_stream2"], np.float32)
    xs_ = [x1[b], x2[b]]
    my, ot = xs_[s], xs_[1 - s]
    x_own = np.ascontiguousarray(my[sub * CH : (sub + 1) * CH])
    x_rest = np.ascontiguousarray(np.stack([
        my[(1 - sub) * CH : (2 - sub) * CH],
        ot[0:CH],
        ot[CH : 2 * CH],
    ]))

    pos = np.concatenate([
        np.arange(s * T + sub * CH, s * T + (sub + 1) * CH),
        np.arange(s * T + (1 - sub) * CH, s * T + (2 - sub) * CH),
        np.arange((1 - s) * T, (1 - s) * T + CH),
        np.arange((1 - s) * T + CH, (1 - s) * T + 2 * CH),
    ])
    inv = (1.0 / (10000.0 ** (np.arange(0, DH, 2, dtype=np.float32) / DH)))
    inv = inv.astype(_BF16).astype(np.float32)
    freqs = pos[:, None].astype(np.float32) * inv[None, :]
    emb = np.concatenate([freqs, freqs], axis=-1)
    cos_d, sin_d = np.cos(emb), np.sin(emb)
    cos_p = np.zeros((128, N), np.float32)
    sin_p = np.zeros((128, N), np.float32)
    cos_p[_ROWS_LO] = cos_d[:, 0:48].T
    cos_p[_ROWS_HI] = cos_d[:, 48:96].T
    sin_p[_ROWS_LO] = sin_d[:, 48:96].T
    sin_p[_ROWS_HI] = sin_d[:, 0:48].T

    qkv_w = [np.asarray(inp["qkv_w"], np.float32), np.asarray(inp["qkv2_w"], np.float32)]
    qkv_b = [np.asarray(inp["qkv_b"], np.float32), np.asarray(inp["qkv2_b"], np.float32)]

    def qkv_part(si, part):
        return qkv_w[si][:, part * D : (part + 1) * D], qkv_b[si][part * D : (part + 1) * D]

    def padded(si, part):
        wfull, bfull = qkv_part(si, part)
        wp = np.zeros((D, H * PH), np.float32)
        bp = np.zeros((128, H), np.float32)
        for h in range(H):
            whp, bhp = _pad_head_cols(wfull[:, h * DH : (h + 1) * DH],
                                      bfull[h * DH : (h + 1) * DH])
            wp[:, h * PH : (h + 1) * PH] = whp
            bp[:, h] = bhp
        return wp, bp

    wq_p, bq_p = padded(s, 0)
    wkm_p, bkm_p = padded(s, 1)
    wko_p, bko_p = padded(1 - s, 1)

    def v_aug(si):
        wfull, bfull = qkv_part(si, 2)
        wa = np.zeros((D, VW), np.float32)
        ba = np.zeros((1, VW), np.float32)
        for h in range(H):
            wa[:, h * 97 : h * 97 + 96] = wfull[:, h * DH : (h + 1) * DH]
            ba[0, h * 97 : h * 97 + 96] = bfull[h * DH : (h + 1) * DH]
            ba[0, h * 97 + 96] = 1.0
        return wa.astype(_BF16), ba

    wvm_a, bvm_a = v_aug(s)
    wvo_a, bvo_a = v_aug(1 - s)

    qs = np.asarray(inp["qk_scale"], np.float32)
    s2 = np.zeros((128, 1), np.float32)
    s2[_ROWS_LO, 0] = qs[0:48] ** 2
    s2[_ROWS_HI, 0] = qs[48:96] ** 2

    def l2cols(v):
        return np.ascontiguousarray(np.asarray(v, np.float32).reshape(KT, 128).T)

    ms_my, mh_my, g_my = (0, 1, 2) if s == 0 else (3, 4, 5)
    ms_ot, mh_ot = (3, 4) if s == 0 else (0, 1)
    m3s, m3h, m3g = (6, 7, 8) if s == 0 else (9, 10, 11)

    w2f = np.asarray(inp["mod_w2"], np.float32)
    b2f = np.asarray(inp["mod_b2"], np.float32)
    cw = lambda i: w2f[:, i * D : (i + 1) * D]
    cb = lambda i: b2f[i * D : (i + 1) * D]
    main_idx = [ms_my, mh_my, ms_ot, mh_ot, m3s, m3h]
    mod_w2m = np.concatenate([cw(i) for i in main_idx], axis=1).astype(_BF16)
    mod_b2m = np.ascontiguousarray(np.concatenate([l2cols(cb(i)) for i in main_idx], axis=1))
    mod_w2g = np.concatenate([cw(g_my), cw(m3g)], axis=1).astype(_BF16)
    mod_b2g = np.ascontiguousarray(np.concatenate([cb(g_my), cb(m3g)])[None, :])

    wo_f = np.asarray(inp["out_w"], np.float32)
    wo_dev = np.ascontiguousarray(wo_f.reshape(H, DH, D).transpose(1, 0, 2).reshape(DH, H * D))

    norm1 = [np.asarray(inp["norm11_w"], np.float32), np.asarray(inp["norm12_w"], np.float32)]
    norm2 = [np.asarray(inp["norm21_w"], np.float32), np.asarray(inp["norm22_w"], np.float32)]
    mlw = [
        (inp["mlp1_w1"], inp["mlp1_b1"], inp["mlp1_w2"], inp["mlp1_b2"]),
        (inp["mlp2_w1"], inp["mlp2_b1"], inp["mlp2_w2"], inp["mlp2_b2"]),
    ]
    w1f, b1f, w2mf, b2mf = [np.asarray(a, np.float32) for a in mlw[s]]

    return {
        "x_own": x_own,
        "x_rest": x_rest,
        "x_own2": x_own.copy(),
        "p_my": np.asarray(inp["p_emb"], np.float32)[b].astype(_BF16),
        "mod_w1": np.asarray(inp["mod_w1"], np.float32).astype(_BF16),
        "mod_b1": np.ascontiguousarray(np.asarray(inp["mod_b1"], np.float32).reshape(4, 128).T),
        "mod_w2m": mod_w2m,
        "mod_b2m": mod_b2m,
        "mod_w2g": mod_w2g,
        "mod_b2g": mod_b2g,
        "norm1_my": l2cols(norm1[s]),
        "norm1_ot": l2cols(norm1[1 - s]),
        "norm2_my": l2cols(norm2[s]),
        "wq": wq_p, "bq": bq_p,
        "wk_my": wkm_p, "bk_my": bkm_p,
        "wk_ot": wko_p, "bk_ot": bko_p,
        "wv_my": wvm_a, "bv_my": bvm_a,
        "wv_ot": wvo_a, "bv_ot": bvo_a,
        "cos_t": cos_p.astype(_BF16), "sin_t": sin_p.astype(_BF16), "qk_s2": s2,
        "wo": wo_dev.astype(_BF16),
        "ob_g": np.ascontiguousarray(np.asarray(inp["out_b"], np.float32)[None, :]),
        "w1": w1f.astype(_BF16),
        "b1c": np.ascontiguousarray(b1f.reshape(MT2, 128).T),
        "w2": w2mf.astype(_BF16),
        "b2r": np.ascontiguousarray(b2mf[None, :]),
    }


def kernel(**inputs):
    nc = _build()
    in_maps = [_prep_core_inputs(c, inputs) for c in range(NC)]
    res = run_bass_kernel_spmd(nc, in_maps, core_ids=list(range(NC)), trace=False)
    out1 = np.zeros((B, T, D), np.float32)
    out2 = np.zeros((B, T, D), np.float32)
    for c in range(NC):
        b, r = c // 4, c % 4
        dst = out1 if r < 2 else out2
        sub = r % 2
        dst[b, sub * CH : (sub + 1) * CH] = res.results[c]["my_out"]
    return out1, out2



# revision 13
# speedup vs baseline: 1.1605x; 1.1605x over previous
"""Trainium2 Bass kernel for nn_DoubleStream_Expert (dense double-stream DiT block).

Sharding (8 cores, no collectives): core c -> batch b = c//4, rank r = c%4.
Each core computes the full K/V projections for its batch (2048 tokens, both
streams), but Q / attention / out-proj / MLP only for its own 512 tokens.
Host slices inputs per core and reassembles the two output streams.

Token chunks are fed in a per-core "slot" order (own chunk, other chunk of my
stream, the two chunks of the other stream) so the SPMD program is identical
across cores; attention is permutation-invariant in keys, and RoPE tables are
permuted on the host to match.

Head dims are padded 96->128 with the rotary halves at rows 0..47 / 64..111,
making rotate_half a uniform +-64 partition move (32-aligned starts, written
via shifted-output ops). Padded weight columns are zero. The rotate sign is
folded into the host sin table so rope is 4 DVE ops per group.

Precision: bf16 matmuls throughout (projections, scores, probs x V, MLP);
fp32 for softmax statistics, norms and residuals. Softmax needs no running
max: QK-norm bounds |logits| <= max(qk_scale)^2/sqrt(dh). Softmax
denominators via DVE reciprocal_approx_fast so the scalar engine streams
pure Exp during attention (no activation-table reloads).
"""

import numpy as np

import concourse.bass as bass  # noqa: F401
import concourse.mybir as mybir
import concourse.tile as tile
from concourse import bacc
from concourse.bass_utils import run_bass_kernel_spmd
from concourse.masks import make_identity

try:
    import ml_dtypes
    _BF16 = ml_dtypes.bfloat16
except ImportError:  # pragma: no cover
    _BF16 = np.float32

F32 = mybir.dt.float32
F32R = mybir.dt.float32r
BF16 = mybir.dt.bfloat16
AF = mybir.ActivationFunctionType
ALU = mybir.AluOpType

B, T, D, H, DH, MLPD = 2, 1024, 768, 8, 96, 3072
N = 2 * T
NC = 8
CH = 512
KT = D // 128        # 6
MT2 = MLPD // 128    # 24
PH = 128
VW = H * 97          # 776
EPS = 1e-6

_ROWS_LO = np.arange(0, 48)
_ROWS_HI = np.arange(64, 112)

_CACHED = {}


def _bc3(ap2d, nh):
    """[P, C] -> [P, nh, C] stride-0 broadcast over a middle axis."""
    return ap2d.unsqueeze(1).broadcast_to([ap2d.shape[0], nh, ap2d.shape[1]])


def _build():
    if "nc" in _CACHED:
        return _CACHED["nc"]

    nc = bacc.Bacc("TRN2", target_bir_lowering=False, debug=False, num_devices=NC)

    def din(name, shape, dt=BF16):
        return nc.dram_tensor(name, list(shape), dt, kind="ExternalInput").ap()

    x_own = din("x_own", [CH, D], F32)
    x_rest = din("x_rest", [3, CH, D], F32)
    x_own2 = din("x_own2", [CH, D], F32)               # second copy for the residual
    p_my = din("p_my", [1, 1024], BF16)
    mod_w1 = din("mod_w1", [1024, 512], BF16)
    mod_b1 = din("mod_b1", [128, 4], F32)
    mod_w2m = din("mod_w2m", [512, 6 * D], BF16)  # ms_my mh_my ms_ot mh_ot m3s m3h
    mod_b2m = din("mod_b2m", [128, 36], F32)
    mod_w2g = din("mod_w2g", [512, 2 * D], BF16)  # g_my, m3g
    mod_b2g = din("mod_b2g", [1, 2 * D], F32)
    norm1_my = din("norm1_my", [128, KT], F32)
    norm1_ot = din("norm1_ot", [128, KT], F32)
    norm2_my = din("norm2_my", [128, KT], F32)
    wq = din("wq", [D, H * PH])
    bq = din("bq", [128, H], F32)
    wk_my = din("wk_my", [D, H * PH])
    wk_ot = din("wk_ot", [D, H * PH])
    bk_my = din("bk_my", [128, H], F32)
    bk_ot = din("bk_ot", [128, H], F32)
    wv_my = din("wv_my", [D, VW], BF16)
    wv_ot = din("wv_ot", [D, VW], BF16)
    cos_t = din("cos_t", [128, N], BF16)
    sin_t = din("sin_t", [128, N], BF16)
    qk_s2 = din("qk_s2", [128, 1], F32)
    wo = din("wo", [96, H * D], BF16)
    ob_g = din("ob_g", [1, D], F32)
    w1 = din("w1", [D, MLPD], BF16)
    b1c = din("b1c", [128, MT2], F32)
    w2 = din("w2", [MLPD, D], BF16)
    b2r = din("b2r", [1, D], F32)

    my_out = nc.dram_tensor("my_out", [CH, D], F32, kind="ExternalOutput").ap()

    with tile.TileContext(nc) as tc:
        persist_cm = tc.tile_pool(name="persist", bufs=1)
        pp = persist_cm.__enter__()

        ident = pp.tile([128, 128], F32, name="ident")
        make_identity(nc, ident[:])
        mod_l2 = pp.tile([128, 36], F32, name="mod_l2")
        g_my_bc = pp.tile([128, D], F32, name="g_my_bc")
        m3g_bc = pp.tile([128, D], F32, name="m3g_bc")
        ob_bc = pp.tile([128, D], F32, name="ob_bc")
        b2_bc = pp.tile([128, D], F32, name="b2_bc")
        w1p = pp.tile([128, KT], F32, name="w1p")
        w2p = pp.tile([128, KT], F32, name="w2p")
        w3p = pp.tile([128, KT], F32, name="w3p")
        s2_sb = pp.tile([128, 1], F32, name="s2_sb")
        bq_sb = pp.tile([128, H], F32, name="bq_sb")
        bkm_sb = pp.tile([128, H], F32, name="bkm_sb")
        bko_sb = pp.tile([128, H], F32, name="bko_sb")
        eps_sb = pp.tile([128, 1], F32, name="eps_sb")
        nc.vector.memset(eps_sb[:], EPS)

        ph1_cm = tc.tile_pool(name="ph1", bufs=2, side="right")
        ph1 = ph1_cm.__enter__()
        ph1s_cm = tc.tile_pool(name="ph1s", bufs=1, side="right")
        ph1s = ph1s_cm.__enter__()
        ph1b_cm = tc.tile_pool(name="ph1b", bufs=2, side="right")
        ph1b = ph1b_cm.__enter__()

        # ---- hoisted: stream x chunks 0/1 + rms stats while mod MLP runs ----
        x_l1s = {}
        rstd4s = {}

        def emit_x_load_stats(sl):
            x_l1 = ph1b.tile([128, 4, D], F32, name="x_l1", tag="x_l1")
            src = x_own if sl == 0 else x_rest[sl - 1]
            nc.sync.dma_start(out=x_l1[:], in_=src.rearrange("(t p) c -> p t c", p=128))
            ssq4 = ph1.tile([128, 4], F32, name="ssq4b", tag="ssq4b")
            for tt in range(4):
                sq = ph1s.tile([128, D], F32, name="sq", tag="sq")
                nc.scalar.activation(sq[:], x_l1[:, tt, :], AF.Square,
                                     accum_out=ssq4[:, tt : tt + 1])
            rstd4 = ph1.tile([128, 4], F32, name="rstd4b", tag="rstd4b")
            nc.scalar.activation(rstd4[:], ssq4[:], AF.Abs_reciprocal_sqrt,
                                 scale=1.0 / D, bias=eps_sb[:])
            for tt in range(4):
                nc.vector.tensor_scalar_mul(x_l1[:, tt, :], x_l1[:, tt, :],
                                            rstd4[:, tt : tt + 1])
            x_l1s[sl] = x_l1

        emit_x_load_stats(0)
        emit_x_load_stats(1)

        # ---------------- modulation MLP ----------------
        with (
            nc.named_scope("mod"),
            tc.tile_pool(name="modw", bufs=1) as mw,
            tc.tile_pool(name="psm", bufs=1, space="PSUM") as psm,
            tc.tile_pool(name="psg", bufs=2, space="PSUM") as psg,
        ):
            p_sb = mw.tile([128, 8], BF16, name="p_sb")
            nc.sync.dma_start(out=p_sb[:], in_=p_my.rearrange("o (j r) -> r (o j)", r=128))
            ps2 = mw.tile([128, 8], BF16, name="ps2")
            nc.scalar.activation(ps2[:], p_sb[:], AF.Silu)

            w1m_sb = mw.tile([128, 8, 512], BF16, name="w1m_sb")
            nc.sync.dma_start(out=w1m_sb[:], in_=mod_w1.rearrange("(k p) m -> p k m", p=128))
            b1m_sb = mw.tile([128, 4], F32, name="b1m_sb")
            nc.sync.dma_start(out=b1m_sb[:], in_=mod_b1)
            h_ps = psm.tile([128, 4], F32, name="h_ps")
            for mt in range(4):
                for kt in range(8):
                    nc.tensor.matmul(
                        h_ps[:, mt : mt + 1],
                        w1m_sb[:, kt, mt * 128 : (mt + 1) * 128],
                        ps2[:, kt : kt + 1],
                        start=(kt == 0), stop=(kt == 7),
                    )
            h_l2 = mw.tile([128, 4], BF16, name="h_l2")
            for mt in range(4):
                nc.scalar.activation(h_l2[:, mt : mt + 1], h_ps[:, mt : mt + 1],
                                     AF.Silu, bias=b1m_sb[:, mt : mt + 1])

            w2m_sb = mw.tile([128, 4, 6 * D], BF16, name="w2m_sb")
            nc.sync.dma_start(out=w2m_sb[:], in_=mod_w2m.rearrange("(k p) m -> p k m", p=128))
            b2m_sb = mw.tile([128, 36], F32, name="b2m_sb")
            nc.sync.dma_start(out=b2m_sb[:], in_=mod_b2m)
            mod_ps = psm.tile([128, 36], F32, name="mod_ps")
            for mt in range(36):
                for kt in range(4):
                    nc.tensor.matmul(
                        mod_ps[:, mt : mt + 1],
                        w2m_sb[:, kt, mt * 128 : (mt + 1) * 128],
                        h_l2[:, kt : kt + 1],
                        start=(kt == 0), stop=(kt == 3),
                    )
            nc.vector.tensor_add(mod_l2[:], mod_ps[:], b2m_sb[:])

            w2g_sb = mw.tile([128, 4, 2 * D], BF16, name="w2g_sb")
            nc.sync.dma_start(out=w2g_sb[:], in_=mod_w2g.rearrange("(k p) m -> p k m", p=128))
            b2g_sb = mw.tile([1, 2 * D], F32, name="b2g_sb")
            nc.sync.dma_start(out=b2g_sb[:], in_=mod_b2g)
            gates = mw.tile([1, 2 * D], F32, name="gates")
            for nt in range(3):
                g_ps = psg.tile([1, 512], F32, name="g_ps", tag="g_ps")
                for kt in range(4):
                    nc.tensor.matmul(
                        g_ps[:], h_l2[:, kt : kt + 1],
                        w2g_sb[:, kt, nt * 512 : (nt + 1) * 512],
                        start=(kt == 0), stop=(kt == 3),
                    )
                nc.vector.tensor_tensor(gates[:, nt * 512 : (nt + 1) * 512], g_ps[:],
                                        b2g_sb[:, nt * 512 : (nt + 1) * 512], op=ALU.add)
            nc.gpsimd.partition_broadcast(g_my_bc[:], gates[:, 0:D])
            nc.gpsimd.partition_broadcast(m3g_bc[:], gates[:, D : 2 * D])

            obg_sb = mw.tile([1, D], F32, name="obg_sb")
            nc.sync.dma_start(out=obg_sb[:], in_=ob_g)
            nc.gpsimd.partition_broadcast(ob_bc[:], obg_sb[:])
            b2r_sb = mw.tile([1, D], F32, name="b2r_sb")
            nc.sync.dma_start(out=b2r_sb[:], in_=b2r)
            nc.gpsimd.partition_broadcast(b2_bc[:], b2r_sb[:])

            n1my_sb = mw.tile([128, KT], F32, name="n1my_sb")
            n1ot_sb = mw.tile([128, KT], F32, name="n1ot_sb")
            n2my_sb = mw.tile([128, KT], F32, name="n2my_sb")
            nc.sync.dma_start(out=n1my_sb[:], in_=norm1_my)
            nc.sync.dma_start(out=n1ot_sb[:], in_=norm1_ot)
            nc.sync.dma_start(out=n2my_sb[:], in_=norm2_my)
            tmp6 = mw.tile([128, KT], F32, name="tmp6")
            nc.vector.tensor_scalar_add(tmp6[:], mod_l2[:, 0:6], 1.0)
            nc.vector.tensor_mul(w1p[:], n1my_sb[:], tmp6[:])
            tmp6b = mw.tile([128, KT], F32, name="tmp6b")
            nc.vector.tensor_scalar_add(tmp6b[:], mod_l2[:, 12:18], 1.0)
            nc.vector.tensor_mul(w2p[:], n1ot_sb[:], tmp6b[:])
            tmp6c = mw.tile([128, KT], F32, name="tmp6c")
            nc.vector.tensor_scalar_add(tmp6c[:], mod_l2[:, 24:30], 1.0)
            nc.vector.tensor_mul(w3p[:], n2my_sb[:], tmp6c[:])
            nc.sync.dma_start(out=s2_sb[:], in_=qk_s2)
            nc.sync.dma_start(out=bq_sb[:], in_=bq)
            nc.sync.dma_start(out=bkm_sb[:], in_=bk_my)
            nc.sync.dma_start(out=bko_sb[:], in_=bk_ot)

        # ---------------- big persistent activations ----------------
        x1n = pp.tile([128, 4, D], F32, name="x1n")
        attnn = pp.tile([96, H, CH], BF16, name="attnn")
        pref = None
        with tc.tile_pool(name="poolA", bufs=1) as pa:
            K_sb = pa.tile([128, H, N], BF16, name="K_sb")
            V_sb = pa.tile([128, N // 128, VW], BF16, name="V_sb")
            Q_sb = pa.tile([128, H, CH], BF16, name="Q_sb")
            # per-head ones columns for the softmax denominators; V evacuation
            # copies never touch these columns, so init once up front.
            nc.vector.memset(V_sb[:, :, 96 : VW : 97], 1.0)

            # ---------------- phase 1: xm + Q/K/V projections + rope ----------------
            with (
                nc.named_scope("proj"),
                tc.tile_pool(name="trig", bufs=1) as trig,
                tc.tile_pool(name="wkvp_q", bufs=1) as wkvp_q,
                tc.tile_pool(name="wkvp_k", bufs=2) as wkvp_k,
                tc.tile_pool(name="wkvp_v", bufs=1) as wkvp_v,
                tc.tile_pool(name="psP", bufs=2, space="PSUM") as psP,
                tc.tile_pool(name="psV", bufs=2, space="PSUM") as psV,
                tc.tile_pool(name="psT", bufs=2, space="PSUM") as psT,
            ):
                cos_sb = trig.tile([128, N], BF16, name="cos_sb")
                sin_sb = trig.tile([128, N], BF16, name="sin_sb")
                nc.sync.dma_start(out=cos_sb[:], in_=cos_t)
                nc.sync.dma_start(out=sin_sb[:], in_=sin_t)

                wq_sb = wkvp_q.tile([128, KT, H * PH], BF16, name="wq_sb")
                nc.sync.dma_start(out=wq_sb[:], in_=wq.rearrange("(k p) m -> p k m", p=128))

                wk_cur = None
                wv_cur = None
                for sl in range(4):
                    my_stream = sl < 2
                    if sl >= 2:
                        emit_x_load_stats(sl)
                    x_l1 = x_l1s.pop(sl)

                    # transpose + modulate -> xm_l2 (bf16), batched per D-chunk
                    xm_l2 = ph1b.tile([128, KT, CH], BF16, name="xm_l2", tag="xm_l2")
                    wsel = w1p if my_stream else w2p
                    hoff = 6 if my_stream else 18
                    for ft in range(KT):
                        tp = psT.tile([128, CH], F32, name="tp", tag="tp")
                        for tt in range(4):
                            nc.tensor.transpose(
                                tp[:, tt * 128 : (tt + 1) * 128],
                                x_l1[:, tt, ft * 128 : (ft + 1) * 128], ident[:])
                        nc.vector.tensor_scalar(
                            xm_l2[:, ft, :], tp[:],
                            wsel[:, ft : ft + 1], mod_l2[:, hoff + ft : hoff + ft + 1],
                            op0=ALU.mult, op1=ALU.add,
                        )

                    # Q projection (own chunk only)
                    if sl == 0:
                        for h in range(H):
                            qp = psP.tile([128, CH], F32, name="qp", tag="qp")
                            for kt in range(KT):
                                nc.tensor.matmul(
                                    qp[:], wq_sb[:, kt, h * PH : (h + 1) * PH],
                                    xm_l2[:, kt, :], start=(kt == 0), stop=(kt == KT - 1),
                                )
                            nc.scalar.activation(Q_sb[:, h, :], qp[:], AF.Identity,
                                                 bias=bq_sb[:, h : h + 1])

                    # K projection
                    if sl in (0, 2):
                        wk_sb = wkvp_k.tile([128, KT, H * PH], BF16, name="wk_sb", tag="wk")
                        nc.sync.dma_start(
                            out=wk_sb[:],
                            in_=(wk_my if my_stream else wk_ot).rearrange("(k p) m -> p k m", p=128),
                        )
                        wk_cur = wk_sb
                    bsel = bkm_sb if my_stream else bko_sb
                    for h in range(H):
                        kp = psP.tile([128, CH], F32, name="kp", tag="qp")
                        for kt in range(KT):
                            nc.tensor.matmul(
                                kp[:], wk_cur[:, kt, h * PH : (h + 1) * PH],
                                xm_l2[:, kt, :], start=(kt == 0), stop=(kt == KT - 1),
                            )
                        nc.scalar.activation(K_sb[:, h, sl * CH : (sl + 1) * CH], kp[:],
                                             AF.Identity, bias=bsel[:, h : h + 1])

                    # V projection, direct L1 (bias folded into out-proj bias on host;
                    # the per-head 97th ones-column is re-set after each evacuation)
                    if sl in (0, 2):
                        wv_sb = wkvp_v.tile([128, KT, VW], BF16, name="wv_sb", tag="wv")
                        nc.sync.dma_start(
                            out=wv_sb[:],
                            in_=(wv_my if my_stream else wv_ot).rearrange("(k p) m -> p k m", p=128),
                        )
                        wv_cur = wv_sb
                    VA = 5 * 97  # split at a head boundary so copies skip ones-columns
                    for tt in range(4):
                        vp1 = psV.tile([128, VA], F32, name="vp1", tag="vp1")
                        vp2 = psV.tile([128, VW - VA], F32, name="vp2", tag="vp2")
                        for kt in range(KT):
                            nc.tensor.matmul(
                                vp1[:], xm_l2[:, kt, tt * 128 : (tt + 1) * 128],
                                wv_cur[:, kt, 0:VA], start=(kt == 0), stop=(kt == KT - 1),
                            )
                        for kt in range(KT):
                            nc.tensor.matmul(
                                vp2[:], xm_l2[:, kt, tt * 128 : (tt + 1) * 128],
                                wv_cur[:, kt, VA:VW], start=(kt == 0), stop=(kt == KT - 1),
                            )
                        vdst = V_sb[:, sl * 4 + tt, :].rearrange("p (h c) -> p h c", h=8)
                        nc.scalar.copy(vdst[:, 0:5, 0:96],
                                       vp1[:].rearrange("p (h c) -> p h c", h=5)[:, :, 0:96])
                        nc.scalar.copy(vdst[:, 5:8, 0:96],
                                       vp2[:].rearrange("p (h c) -> p h c", h=3)[:, :, 0:96])

                    # rope on this K chunk (sign folded into sin table: 4 ops/group)
                    c3 = cos_sb[:, sl * CH : (sl + 1) * CH]
                    s3 = sin_sb[:, sl * CH : (sl + 1) * CH]
                    HG = H // 2
                    for hg in range(2):
                        kr_t = ph1s.tile([128, HG, CH], BF16, name="kr_t", tag="kr_t")
                        kr_m = ph1s.tile([128, HG, CH], BF16, name="kr_m", tag="kr_m")
                        ksl = K_sb[:, hg * HG : (hg + 1) * HG, sl * CH : (sl + 1) * CH]
                        nc.vector.tensor_tensor(kr_t[:], ksl, _bc3(c3, HG), op=ALU.mult)
                        nc.vector.tensor_tensor(kr_m[0:64], ksl[64:128], _bc3(s3[64:128], HG), op=ALU.mult)
                        nc.vector.tensor_tensor(kr_m[64:128], ksl[0:64], _bc3(s3[0:64], HG), op=ALU.mult)
                        nc.vector.tensor_tensor(ksl, kr_t[:], kr_m[:], op=ALU.add)

                    if sl == 0:
                        c0 = cos_sb[:, 0:CH]
                        s0 = sin_sb[:, 0:CH]
                        for hg in range(2):
                            qr_t = ph1s.tile([128, HG, CH], BF16, name="qr_t", tag="kr_t")
                            qr_m = ph1s.tile([128, HG, CH], BF16, name="qr_m", tag="kr_m")
                            qsl = Q_sb[:, hg * HG : (hg + 1) * HG, :]
                            nc.vector.tensor_tensor(qr_t[:], qsl, _bc3(c0, HG), op=ALU.mult)
                            nc.vector.tensor_tensor(qr_m[0:64], qsl[64:128], _bc3(s0[64:128], HG), op=ALU.mult)
                            nc.vector.tensor_tensor(qr_m[64:128], qsl[0:64], _bc3(s0[0:64], HG), op=ALU.mult)
                            nc.vector.tensor_tensor(qsl, qr_t[:], qr_m[:], op=ALU.add)

            ph1b_cm.__exit__(None, None, None)
            ph1s_cm.__exit__(None, None, None)
            ph1_cm.__exit__(None, None, None)

            # ---------------- phases 2+3: qk-norm, attention, out-proj, residual ----------------
            # "pref" outlives poolA (holds out-proj + MLP weights prefetched
            # during attention, consumed through phase 4); released at the end.
            pref_cm = tc.tile_pool(name="pref", bufs=1, side="right")
            pref = pref_cm.__enter__()
            with (
                tc.tile_pool(name="ph2", bufs=2) as ph2,
                tc.tile_pool(name="php", bufs=3) as php,
                tc.tile_pool(name="ph2s", bufs=1) as ph2s,
            ):
                # prefetch out-proj weights + first MLP weight during attention
                wo_sb = pref.tile([96, H, D], BF16, name="wo_sb")
                nc.sync.dma_start(out=wo_sb[:], in_=wo.rearrange("p (h m) -> p h m", h=H))
                xo_l1 = pref.tile([128, 4, D], F32, name="xo_l1")
                nc.sync.dma_start(out=xo_l1[:], in_=x_own2.rearrange("(t p) c -> p t c", p=128))
                w1_sb = pref.tile([128, KT, MLPD], BF16, name="w1_sb")
                nc.sync.dma_start(out=w1_sb[:], in_=w1.rearrange("(k p) m -> p k m", p=128))
                b1_sb = pref.tile([128, MT2], F32, name="b1_sb")
                nc.sync.dma_start(out=b1_sb[:], in_=b1c)

                with (
                    nc.named_scope("attn"),
                    tc.tile_pool(name="psK", bufs=1, space="PSUM") as psK,
                    tc.tile_pool(name="psS", bufs=3, space="PSUM") as psS,
                    tc.tile_pool(name="psPV", bufs=2, space="PSUM") as psPV,
                ):
                    ones_bf = ph2s.tile([128, 1], BF16, name="ones_bf")
                    nc.vector.memset(ones_bf[:], 1.0)

                    # rk_all[kt-token, h*16+kt2] = 1/(sqrt(dh)*|k|), per-partition layout
                    rk_all = ph2s.tile([128, H * 16], F32, name="rk_all")
                    rk_ps = psK.tile([128, H * 16], F32, name="rk_ps", tag="rk_ps")
                    for h in range(H):
                        ksq = ph2.tile([128, N], BF16, name="ksq", tag="ksq")
                        nc.vector.tensor_mul(ksq[:], K_sb[:, h, :], K_sb[:, h, :])
                        for kt2 in range(16):
                            nc.tensor.matmul(
                                rk_ps[:, h * 16 + kt2 : h * 16 + kt2 + 1],
                                ksq[:, kt2 * 128 : (kt2 + 1) * 128],
                                ones_bf[:], start=True, stop=True,
                            )
                    nc.scalar.activation(rk_all[:], rk_ps[:], AF.Abs_reciprocal_sqrt,
                                         scale=float(DH), bias=eps_sb[:])

                    # q_hat = q * s2 * (1/|q|)
                    for h in range(H):
                        qsq = ph2.tile([128, CH], BF16, name="qsq", tag="qsq")
                        nc.vector.tensor_mul(qsq[:], Q_sb[:, h, :], Q_sb[:, h, :])
                        rq_ps = psK.tile([1, CH], F32, name="rq_ps", tag="rq_ps")
                        nc.tensor.matmul(rq_ps[:], ones_bf[:], qsq[:], start=True, stop=True)
                        rq_bf = ph2.tile([1, CH], BF16, name="rq_bf", tag="rq_bf")
                        nc.scalar.activation(rq_bf[:], rq_ps[:], AF.Abs_reciprocal_sqrt,
                                             bias=eps_sb[0:1, :])
                        rq_bc = ph2.tile([128, CH], BF16, name="rq_bc", tag="rq_bc")
                        nc.gpsimd.partition_broadcast(rq_bc[:], rq_bf[:])
                        nc.vector.scalar_tensor_tensor(
                            Q_sb[:, h, :], Q_sb[:, h, :], s2_sb[:], rq_bc[:],
                            op0=ALU.mult, op1=ALU.mult,
                        )

                    # flattened (head, key-tile) stream: scores 2 tiles ahead of
                    # exp/PV so the PE never stalls on the scalar engine.
                    seq = [(h, k) for h in range(H) for k in range(16)]
                    sps_tiles = {}

                    def emit_score(j):
                        h, k = seq[j]
                        sp = psS.tile([128, CH], F32, name="sps", tag="sps")
                        nc.tensor.matmul(
                            sp[:], K_sb[:, h, k * 128 : (k + 1) * 128],
                            Q_sb[:, h, :], start=True, stop=True,
                        )
                        sps_tiles[j] = sp

                    emit_score(0)
                    emit_score(1)
                    pv = None
                    for j, (h, k) in enumerate(seq):
                        if k == 0:
                            pv = psPV.tile([128, CH], F32, name="pv", tag="pv")
                        pt = php.tile([128, CH], BF16, name="pt", tag="pt")
                        nc.scalar.activation(pt[:], sps_tiles.pop(j)[:], AF.Exp,
                                             scale=rk_all[:, h * 16 + k : h * 16 + k + 1])
                        if j + 2 < len(seq):
                            emit_score(j + 2)
                        nc.tensor.matmul(
                            pv[0:97, :], V_sb[:, k, h * 97 : (h + 1) * 97],
                            pt[:], start=(k == 0), stop=(k == 15),
                        )
                        if k == 15:
                            # denominator: rs = 1/sum(exp) on DVE (keeps ACT pure-Exp)
                            dn_row = ph2.tile([1, CH], F32, name="dn_row", tag="dn_row")
                            nc.vector.tensor_copy(dn_row[:], pv[96:97, :])
                            rs_row = ph2.tile([1, CH], F32, name="rs_row", tag="rs_row")
                            nc.vector.reciprocal_approx_fast(rs_row[:], dn_row[:])
                            rs_bc = ph2.tile([96, CH], F32, name="rs_bc", tag="rs_bc")
                            nc.gpsimd.partition_broadcast(rs_bc[:], rs_row[:], channels=96)
                            nc.vector.tensor_tensor(attnn[:, h, :], pv[0:96, :], rs_bc[:],
                                                    op=ALU.mult)

                # out-proj + residual
                with (
                    nc.named_scope("oproj"),
                    tc.tile_pool(name="psO", bufs=2, space="PSUM") as psO,
                ):
                    for qt in range(4):
                        op1 = psO.tile([128, 512], F32, name="op1", tag="op1")
                        op2 = psO.tile([128, D - 512], F32, name="op2", tag="op2")
                        for h in range(H):
                            nc.tensor.matmul(
                                op1[:], attnn[:, h, qt * 128 : (qt + 1) * 128],
                                wo_sb[:, h, 0:512], start=(h == 0), stop=(h == H - 1),
                            )
                        for h in range(H):
                            nc.tensor.matmul(
                                op2[:], attnn[:, h, qt * 128 : (qt + 1) * 128],
                                wo_sb[:, h, 512:D], start=(h == 0), stop=(h == H - 1),
                            )
                        t1 = ph2.tile([128, D], F32, name="t1", tag="t1")
                        nc.vector.tensor_tensor(t1[:, 0:512], op1[:], ob_bc[:, 0:512], op=ALU.add)
                        nc.vector.tensor_tensor(t1[:, 512:D], op2[:], ob_bc[:, 512:D], op=ALU.add)
                        nc.vector.tensor_mul(t1[:], t1[:], g_my_bc[:])
                        nc.vector.tensor_tensor(x1n[:, qt, :], t1[:], xo_l1[:, qt, :], op=ALU.add)

        # ---------------- phase 4: norm2 + MLP + final ----------------
        with (
            nc.named_scope("mlp"),
            tc.tile_pool(name="mlpw", bufs=1) as mlpw,
            tc.tile_pool(name="ph4", bufs=2) as ph4,
            tc.tile_pool(name="psM", bufs=2, space="PSUM") as psM,
            tc.tile_pool(name="psM2", bufs=2, space="PSUM") as psM2,
            tc.tile_pool(name="psT2", bufs=2, space="PSUM") as psT2,
        ):
            w2_sb = mlpw.tile([128, MT2, D], BF16, name="w2_sb")
            nc.sync.dma_start(out=w2_sb[:], in_=w2.rearrange("(k p) m -> p k m", p=128))
            xn_l2 = mlpw.tile([128, KT, CH], BF16, name="xn_l2")
            ssq4m = ph4.tile([128, 4], F32, name="ssq4m", tag="ssq4m")
            for tt in range(4):
                sq = ph4.tile([128, D], F32, name="sq4", tag="sq4")
                nc.scalar.activation(sq[:], x1n[:, tt, :], AF.Square,
                                     accum_out=ssq4m[:, tt : tt + 1])
            rstd4m = ph4.tile([128, 4], F32, name="rstd4m", tag="rstd4m")
            nc.scalar.activation(rstd4m[:], ssq4m[:], AF.Abs_reciprocal_sqrt,
                                 scale=1.0 / D, bias=eps_sb[:])
            xs_all = mlpw.tile([128, 4, D], F32, name="xs_all")
            for tt in range(4):
                nc.vector.tensor_scalar_mul(xs_all[:, tt, :], x1n[:, tt, :],
                                            rstd4m[:, tt : tt + 1])
            for ft in range(KT):
                tp = psT2.tile([128, CH], F32, name="tp2", tag="tp2")
                for tt in range(4):
                    nc.tensor.transpose(
                        tp[:, tt * 128 : (tt + 1) * 128],
                        xs_all[:, tt, ft * 128 : (ft + 1) * 128], ident[:])
                nc.vector.tensor_scalar(
                    xn_l2[:, ft, :], tp[:],
                    w3p[:, ft : ft + 1], mod_l2[:, 30 + ft : 30 + ft + 1],
                    op0=ALU.mult, op1=ALU.add,
                )

            h_bf = mlpw.tile([128, MT2, CH], BF16, name="h_bf")
            for mt in range(MT2):
                fp = psM.tile([128, CH], F32, name="fp", tag="fp")
                for kt in range(KT):
                    nc.tensor.matmul(
                        fp[:], w1_sb[:, kt, mt * 128 : (mt + 1) * 128],
                        xn_l2[:, kt, :], start=(kt == 0), stop=(kt == KT - 1),
                    )
                nc.scalar.activation(h_bf[:, mt, :], fp[:], AF.Gelu,
                                     bias=b1_sb[:, mt : mt + 1])

            out_f = mlpw.tile([128, 4, D], F32, name="out_f")
            for qt in range(4):
                f1 = psM2.tile([128, 512], F32, name="f1", tag="f1")
                f2 = psM2.tile([128, D - 512], F32, name="f2", tag="f2")
                for mt in range(MT2):
                    nc.tensor.matmul(
                        f1[:], h_bf[:, mt, qt * 128 : (qt + 1) * 128],
                        w2_sb[:, mt, 0:512], start=(mt == 0), stop=(mt == MT2 - 1),
                    )
                for mt in range(MT2):
                    nc.tensor.matmul(
                        f2[:], h_bf[:, mt, qt * 128 : (qt + 1) * 128],
                        w2_sb[:, mt, 512:D], start=(mt == 0), stop=(mt == MT2 - 1),
                    )
                t2 = ph4.tile([128, D], F32, name="t2", tag="t2")
                nc.vector.tensor_tensor(t2[:, 0:512], f1[:], b2_bc[:, 0:512], op=ALU.add)
                nc.vector.tensor_tensor(t2[:, 512:D], f2[:], b2_bc[:, 512:D], op=ALU.add)
                nc.vector.tensor_mul(t2[:], t2[:], m3g_bc[:])
                nc.vector.tensor_tensor(out_f[:, qt, :], t2[:], x1n[:, qt, :], op=ALU.add)
            nc.sync.dma_start(out=my_out.rearrange("(t p) c -> p t c", p=128), in_=out_f[:])

        pref_cm.__exit__(None, None, None)
        persist_cm.__exit__(None, None, None)


    nc.compile()
    _CACHED["nc"] = nc
    return nc


def _pad_head_cols(w_h, b_h):
    wp = np.zeros((D, PH), np.float32)
    bp = np.zeros((PH,), np.float32)
    wp[:, _ROWS_LO] = w_h[:, 0:48]
    wp[:, _ROWS_HI] = w_h[:, 48:96]
    bp[_ROWS_LO] = b_h[0:48]
    bp[_ROWS_HI] = b_h[48:96]
    return wp, bp


def _prep_core_inputs(c, inp):
    b, r = c // 4, c % 4
    s = 0 if r < 2 else 1
    sub = r % 2

    x1 = np.asarray(inp["x_stream1"], np.float32)
    x2 = np.asarray(inp["x_stream2"], np.float32)
    xs_ = [x1[b], x2[b]]
    my, ot = xs_[s], xs_[1 - s]
    x_own = np.ascontiguousarray(my[sub * CH : (sub + 1) * CH])
    x_rest = np.ascontiguousarray(np.stack([
        my[(1 - sub) * CH : (2 - sub) * CH],
        ot[0:CH],
        ot[CH : 2 * CH],
    ]))

    pos = np.concatenate([
        np.arange(s * T + sub * CH, s * T + (sub + 1) * CH),
        np.arange(s * T + (1 - sub) * CH, s * T + (2 - sub) * CH),
        np.arange((1 - s) * T, (1 - s) * T + CH),
        np.arange((1 - s) * T + CH, (1 - s) * T + 2 * CH),
    ])
    inv = (1.0 / (10000.0 ** (np.arange(0, DH, 2, dtype=np.float32) / DH)))
    inv = inv.astype(_BF16).astype(np.float32)
    freqs = pos[:, None].astype(np.float32) * inv[None, :]
    emb = np.concatenate([freqs, freqs], axis=-1)
    cos_d, sin_d = np.cos(emb), np.sin(emb)
    cos_p = np.zeros((128, N), np.float32)
    sin_p = np.zeros((128, N), np.float32)
    cos_p[_ROWS_LO] = cos_d[:, 0:48].T
    cos_p[_ROWS_HI] = cos_d[:, 48:96].T
    sin_p[_ROWS_LO] = sin_d[:, 48:96].T
    sin_p[_ROWS_HI] = -sin_d[:, 0:48].T   # rotate-half sign folded into the table

    qkv_w = [np.asarray(inp["qkv_w"], np.float32), np.asarray(inp["qkv2_w"], np.float32)]
    qkv_b = [np.asarray(inp["qkv_b"], np.float32), np.asarray(inp["qkv2_b"], np.float32)]

    def qkv_part(si, part):
        return qkv_w[si][:, part * D : (part + 1) * D], qkv_b[si][part * D : (part + 1) * D]

    def padded(si, part):
        wfull, bfull = qkv_part(si, part)
        wp = np.zeros((D, H * PH), np.float32)
        bp = np.zeros((128, H), np.float32)
        for h in range(H):
            whp, bhp = _pad_head_cols(wfull[:, h * DH : (h + 1) * DH],
                                      bfull[h * DH : (h + 1) * DH])
            wp[:, h * PH : (h + 1) * PH] = whp
            bp[:, h] = bhp
        return wp, bp

    wq_p, bq_p = padded(s, 0)
    wkm_p, bkm_p = padded(s, 1)
    wko_p, bko_p = padded(1 - s, 1)

    def v_aug(si):
        wfull, _ = qkv_part(si, 2)
        wa = np.zeros((D, VW), np.float32)
        for h in range(H):
            wa[:, h * 97 : h * 97 + 96] = wfull[:, h * DH : (h + 1) * DH]
        return wa.astype(_BF16)

    wvm_a = v_aug(s)
    wvo_a = v_aug(1 - s)

    # V bias folded through the out-projection (valid because both streams
    # share the same v-bias vector; asserted below).
    vb_my = qkv_b[s][2 * D : 3 * D]
    vb_ot = qkv_b[1 - s][2 * D : 3 * D]
    assert np.allclose(vb_my, vb_ot), "v-bias fold requires equal stream biases"
    out_w_f = np.asarray(inp["out_w"], np.float32)
    ob_eff = np.asarray(inp["out_b"], np.float32) + vb_my @ out_w_f

    qs = np.asarray(inp["qk_scale"], np.float32)
    s2 = np.zeros((128, 1), np.float32)
    s2[_ROWS_LO, 0] = qs[0:48] ** 2
    s2[_ROWS_HI, 0] = qs[48:96] ** 2

    def l2cols(v):
        return np.ascontiguousarray(np.asarray(v, np.float32).reshape(KT, 128).T)

    ms_my, mh_my, g_my = (0, 1, 2) if s == 0 else (3, 4, 5)
    ms_ot, mh_ot = (3, 4) if s == 0 else (0, 1)
    m3s, m3h, m3g = (6, 7, 8) if s == 0 else (9, 10, 11)

    w2f = np.asarray(inp["mod_w2"], np.float32)
    b2f = np.asarray(inp["mod_b2"], np.float32)
    cw = lambda i: w2f[:, i * D : (i + 1) * D]
    cb = lambda i: b2f[i * D : (i + 1) * D]
    main_idx = [ms_my, mh_my, ms_ot, mh_ot, m3s, m3h]
    mod_w2m = np.concatenate([cw(i) for i in main_idx], axis=1).astype(_BF16)
    mod_b2m = np.ascontiguousarray(np.concatenate([l2cols(cb(i)) for i in main_idx], axis=1))
    mod_w2g = np.concatenate([cw(g_my), cw(m3g)], axis=1).astype(_BF16)
    mod_b2g = np.ascontiguousarray(np.concatenate([cb(g_my), cb(m3g)])[None, :])

    wo_dev = np.ascontiguousarray(out_w_f.reshape(H, DH, D).transpose(1, 0, 2).reshape(DH, H * D))

    norm1 = [np.asarray(inp["norm11_w"], np.float32), np.asarray(inp["norm12_w"], np.float32)]
    norm2 = [np.asarray(inp["norm21_w"], np.float32), np.asarray(inp["norm22_w"], np.float32)]
    mlw = [
        (inp["mlp1_w1"], inp["mlp1_b1"], inp["mlp1_w2"], inp["mlp1_b2"]),
        (inp["mlp2_w1"], inp["mlp2_b1"], inp["mlp2_w2"], inp["mlp2_b2"]),
    ]
    w1f, b1f, w2mf, b2mf = [np.asarray(a, np.float32) for a in mlw[s]]

    return {
        "x_own": x_own,
        "x_rest": x_rest,
        "x_own2": x_own.copy(),
        "p_my": np.asarray(inp["p_emb"], np.float32)[b].astype(_BF16),
        "mod_w1": np.asarray(inp["mod_w1"], np.float32).astype(_BF16),
        "mod_b1": np.ascontiguousarray(np.asarray(inp["mod_b1"], np.float32).reshape(4, 128).T),
        "mod_w2m": mod_w2m,
        "mod_b2m": mod_b2m,
        "mod_w2g": mod_w2g,
        "mod_b2g": mod_b2g,
        "norm1_my": l2cols(norm1[s]),
        "norm1_ot": l2cols(norm1[1 - s]),
        "norm2_my": l2cols(norm2[s]),
        "wq": wq_p.astype(_BF16), "bq": bq_p,
        "wk_my": wkm_p.astype(_BF16), "bk_my": bkm_p,
        "wk_ot": wko_p.astype(_BF16), "bk_ot": bko_p,
        "wv_my": wvm_a,
        "wv_ot": wvo_a,
        "cos_t": cos_p.astype(_BF16), "sin_t": sin_p.astype(_BF16), "qk_s2": s2,
        "wo": wo_dev.astype(_BF16),
        "ob_g": np.ascontiguousarray(ob_eff[None, :]),
        "w1": w1f.astype(_BF16),
        "b1c": np.ascontiguousarray(b1f.reshape(MT2, 128).T),
        "w2": w2mf.astype(_BF16),
        "b2r": np.ascontiguousarray(b2mf[None, :]),
    }


def kernel(**inputs):
    nc = _build()
    in_maps = [_prep_core_inputs(c, inputs) for c in range(NC)]
    res = run_bass_kernel_spmd(nc, in_maps, core_ids=list(range(NC)), trace=False)
    out1 = np.zeros((B, T, D), np.float32)
    out2 = np.zeros((B, T, D), np.float32)
    for c in range(NC):
        b, r = c // 4, c % 4
        dst = out1 if r < 2 else out2
        sub = r % 2
        dst[b, sub * CH : (sub + 1) * CH] = res.results[c]["my_out"]
    return out1, out2
